# revision 3
# baseline (speedup 1.0000x reference)
# Trainium2 Bass kernel for nn_LocalAggregator (Gaussian -> voxel-grid semantic
# compositing).  Strategy: data-parallel over the N=129600 query points (8 cores
# x 16200 points).  Each core's slab is split into 45 groups of 360 points (10
# z-columns of 36).  For every group the host builds the exact list of Gaussians
# whose integer box overlaps the group's (x, y) column rectangle; the quadratic
# form, log-opacity, and the x/y/z integer box tests (as one-hot penalty rows)
# are all folded into a single fp32 matmul
#     E[g, n] = coef[56, G].T @ feat[56, 360]
# followed by Exp on the scalar engine and an fp16 matmul with the semantics to
# produce the [13, 360] output block.  No per-pair vector ops at all.
import numpy as np
import ml_dtypes

H, W, D = 60, 60, 36
GRID = 0.08
SCALE_MULT = 3.0
P = 2048
C = 13
N = H * W * D                  # 129600
NCORES = 8
NPC = N // NCORES              # 16200 points per core
GCOLS = 10                     # columns per group
GPTS = GCOLS * D               # 360 points per group
NG = NPC // GPTS               # 45 groups per core
KF = 10 + GCOLS + D            # 56 feature rows
PEN = -2000.0                  # box-miss penalty (exp(PEN) == 0 in fp32)

_NC_CACHE: dict = {}
_JIT_CACHE: dict = {}


def _build_nc(L_slots):
    import concourse.bacc as bacc
    import concourse.tile as tile
    from concourse import mybir

    Ltot = int(np.sum(L_slots))
    offs = np.concatenate([[0], np.cumsum(L_slots)]).astype(int)

    nc = bacc.Bacc("TRN2", target_bir_lowering=False, debug=False,
                   num_devices=NCORES)
    f32 = mybir.dt.float32
    f16 = mybir.dt.float16
    RHS = nc.dram_tensor("RHS", [KF, NPC], f32, kind="ExternalInput")
    COEF = nc.dram_tensor("COEF", [KF, Ltot], f32, kind="ExternalInput")
    SEM = nc.dram_tensor("SEM", [Ltot, C], f16, kind="ExternalInput")
    OUT = nc.dram_tensor("OUT", [C, NPC], f32, kind="ExternalOutput")

    with tile.TileContext(nc) as tc:
        with (
            tc.tile_pool(name="rhs", bufs=3) as rhs_pool,
            tc.tile_pool(name="coef", bufs=4) as coef_pool,
            tc.tile_pool(name="sem", bufs=4) as sem_pool,
            tc.tile_pool(name="w", bufs=4) as w_pool,
            tc.tile_pool(name="outc", bufs=3) as out_pool,
            tc.tile_pool(name="psE", bufs=4, space="PSUM") as pse_pool,
            tc.tile_pool(name="psO", bufs=2, space="PSUM") as pso_pool,
        ):
            for g in range(NG):
                rhs_t = rhs_pool.tile([KF, GPTS], f32)
                nc.sync.dma_start(rhs_t[:], RHS[:, g * GPTS:(g + 1) * GPTS])
                psO = pso_pool.tile([C, GPTS], f32)
                L = int(L_slots[g])
                tiles = [(int(offs[g]) + s, min(128, L - s))
                         for s in range(0, L, 128)]
                for ti, (off, Mt) in enumerate(tiles):
                    coef_t = coef_pool.tile([KF, Mt], f32)
                    nc.sync.dma_start(coef_t[:], COEF[:, off:off + Mt])
                    psE = pse_pool.tile([Mt, GPTS], f32)
                    nc.tensor.matmul(psE[:], coef_t[:], rhs_t[:],
                                     start=True, stop=True)
                    w_t = w_pool.tile([Mt, GPTS], f16)
                    nc.scalar.activation(w_t[:], psE[:],
                                         mybir.ActivationFunctionType.Exp)
                    sem_t = sem_pool.tile([Mt, C], f16)
                    nc.sync.dma_start(sem_t[:], SEM[off:off + Mt, :])
                    nc.tensor.matmul(psO[:], sem_t[:], w_t[:],
                                     start=(ti == 0),
                                     stop=(ti == len(tiles) - 1))
                out_t = out_pool.tile([C, GPTS], f32)
                nc.vector.tensor_copy(out_t[:], psO[:])
                nc.sync.dma_start(OUT[:, g * GPTS:(g + 1) * GPTS], out_t[:])
    nc.compile()
    return nc


def _get_nc(L_slots):
    key = tuple(int(x) for x in L_slots)
    if key not in _NC_CACHE:
        _NC_CACHE[key] = _build_nc(L_slots)
    return _NC_CACHE[key]


def _get_runner(nc):
    """Cached shard_map-jitted executor for one Bass program (axon/PJRT path).

    Mirrors concourse.bass2jax.run_bass_via_pjrt but keeps the jitted callable
    so repeated runs don't rebuild/recompile."""
    if id(nc) in _JIT_CACHE:
        return _JIT_CACHE[id(nc)]
    import jax
    from concourse import bass2jax, mybir
    from jax.experimental.shard_map import shard_map
    from jax.sharding import Mesh, PartitionSpec

    bass2jax.install_neuronx_cc_hook()
    partition_name = (nc.partition_id_tensor.name
                      if nc.partition_id_tensor else None)
    in_names, out_names, out_avals, zero_outs = [], [], [], []
    for alloc in nc.m.functions[0].allocations:
        if not isinstance(alloc, mybir.MemoryLocationSet):
            continue
        name = alloc.memorylocations[0].name
        if alloc.kind == "ExternalInput":
            if name == partition_name:
                continue
            in_names.append(name)
        elif alloc.kind == "ExternalOutput":
            shape = tuple(alloc.tensor_shape)
            dtype = mybir.dt.np(alloc.dtype)
            out_names.append(name)
            out_avals.append(jax.core.ShapedArray(shape, dtype))
            zero_outs.append(np.zeros(shape, dtype))
    n_params = len(in_names)
    all_in_names = in_names + out_names
    if partition_name is not None:
        all_in_names = all_in_names + [partition_name]

    def _body(*args):
        operands = list(args)
        if partition_name is not None:
            operands.append(bass2jax.partition_id_tensor())
        outs = bass2jax._bass_exec_p.bind(
            *operands,
            out_avals=tuple(out_avals),
            in_names=tuple(all_in_names),
            out_names=tuple(out_names),
            lowering_input_output_aliases=(),
            sim_require_finite=True,
            sim_require_nnan=True,
            nc=nc,
        )
        return tuple(outs)

    devices = jax.devices()[:NCORES]
    mesh = Mesh(np.asarray(devices), ("core",))
    donate = tuple(range(n_params, n_params + len(out_names)))
    sharded = jax.jit(
        shard_map(_body, mesh=mesh,
                  in_specs=(PartitionSpec("core"),) * (n_params + len(out_names)),
                  out_specs=(PartitionSpec("core"),) * len(out_names),
                  check_rep=False),
        donate_argnums=donate, keep_unused=True)

    def run(in_maps, rounds=1):
        concat_in = [np.concatenate([np.asarray(m[nm]) for m in in_maps], axis=0)
                     for nm in in_names]
        outs = None
        for _ in range(rounds):
            zo = [np.concatenate([z] * NCORES, axis=0) for z in zero_outs]
            outs = sharded(*concat_in, *zo)
        outs = [np.asarray(o) for o in outs]
        results = []
        for ci in range(NCORES):
            d = {}
            for oi, nm in enumerate(out_names):
                per = outs[oi].shape[0] // NCORES
                d[nm] = outs[oi][ci * per:(ci + 1) * per]
            results.append(d)
        return results, sharded, (concat_in, zero_outs, in_names, out_names)

    _JIT_CACHE[id(nc)] = run
    return run


def _host_prep(pts, means3D, opacities, semantics, scales, cov3D, origin_use):
    pts = np.asarray(pts, np.float32).reshape(N, 3)
    mu32 = np.asarray(means3D, np.float32).reshape(P, 3)
    op = np.asarray(opacities, np.float64).reshape(P)
    sem = np.asarray(semantics, np.float32).reshape(P, C)
    sc32 = np.asarray(scales, np.float32).reshape(P, 3)
    cov = np.asarray(cov3D, np.float64).reshape(P, 3, 3)
    org32 = np.asarray(origin_use, np.float32).reshape(3)

    # --- integer binning, replicated in fp32 exactly like the reference ---
    radii = np.ceil(sc32.max(-1) * np.float32(SCALE_MULT) / np.float32(GRID)
                    ).astype(np.int32).astype(np.int64)
    m_int = ((mu32 - org32) / np.float32(GRID)).astype(np.int32).astype(np.int64)
    p_int = ((pts - org32) / np.float32(GRID)).astype(np.int32).astype(np.int64)

    # structured-input check: points must be the (i, j, k) voxel-center grid
    idx = np.arange(N)
    kk = idx % D
    col = idx // D
    jj = col % W
    ii = col // W
    grid_int = np.stack([ii, jj, kk], axis=-1)
    if not np.array_equal(p_int, grid_int):
        raise RuntimeError("kernel: unstructured pts not supported by fast path")

    # --- per-Gaussian E coefficients (float64 for the inverse, cast to f32) ---
    a, b, c_, d, e, f = (cov[:, 0, 0], cov[:, 1, 1], cov[:, 2, 2],
                         cov[:, 0, 1], cov[:, 1, 2], cov[:, 0, 2])
    det = a * (b * c_ - e * e) - d * (d * c_ - e * f) + f * (d * e - b * f)
    ixx = (b * c_ - e * e) / det
    iyy = (a * c_ - f * f) / det
    izz = (a * b - d * d) / det
    ixy = (e * f - d * c_) / det
    iyz = (d * f - a * e) / det
    ixz = (d * e - b * f) / det
    A = np.empty((P, 3, 3))
    A[:, 0, 0], A[:, 1, 1], A[:, 2, 2] = ixx, iyy, izz
    A[:, 0, 1] = A[:, 1, 0] = ixy
    A[:, 1, 2] = A[:, 2, 1] = iyz
    A[:, 0, 2] = A[:, 2, 0] = ixz
    mu = mu32.astype(np.float64)
    Amu = np.einsum('pij,pj->pi', A, mu)
    muAmu = np.einsum('pi,pi->p', mu, Amu)
    coef10 = np.stack([
        -0.5 * ixx, -0.5 * iyy, -0.5 * izz,
        -ixy, -iyz, -ixz,
        Amu[:, 0], Amu[:, 1], Amu[:, 2],
        -0.5 * muAmu + np.log(op),
    ]).astype(np.float32)                                    # [10, P]

    # --- per-core RHS feature matrices ---
    x, y, z = pts[:, 0], pts[:, 1], pts[:, 2]
    feat10 = np.stack([x * x, y * y, z * z, x * y, y * z, x * z,
                       x, y, z, np.ones_like(x)])            # [10, N] f32
    nloc = np.arange(NPC)
    cg = (nloc % GPTS) // D
    kz = nloc % D
    onehot = np.zeros((GCOLS + D, NPC), np.float32)
    onehot[cg, nloc] = 1.0
    onehot[GCOLS + kz, nloc] = 1.0
    rhs_all = []
    for ci in range(NCORES):
        r = np.empty((KF, NPC), np.float32)
        r[:10] = feat10[:, ci * NPC:(ci + 1) * NPC]
        r[10:] = onehot
        rhs_all.append(r)

    # --- per-(core, group) Gaussian hit lists and penalty tables ---
    mx, my, mz = m_int[:, 0], m_int[:, 1], m_int[:, 2]
    hits_cg = [[None] * NG for _ in range(NCORES)]
    for ci in range(NCORES):
        for g in range(NG):
            col0 = ci * (NPC // D) + g * GCOLS
            cols = col0 + np.arange(GCOLS)
            gi, gj = cols // W, cols % W
            ox = (mx + radii >= gi.min()) & (mx - radii <= gi.max())
            oy = (my + radii >= gj.min()) & (my - radii <= gj.max())
            hits_cg[ci][g] = np.where(ox & oy)[0]
    L_slots = [max(1, max(len(hits_cg[ci][g]) for ci in range(NCORES)))
               for g in range(NG)]
    Ltot = int(np.sum(L_slots))
    offs = np.concatenate([[0], np.cumsum(L_slots)]).astype(int)

    in_maps = []
    for ci in range(NCORES):
        coef_m = np.zeros((KF, Ltot), np.float32)
        sem_m = np.zeros((Ltot, C), ml_dtypes.float16 if False else np.float16)
        for g in range(NG):
            hit = hits_cg[ci][g]
            nh = len(hit)
            if nh == 0:
                continue
            o = offs[g]
            coef_m[:10, o:o + nh] = coef10[:, hit]
            col0 = ci * (NPC // D) + g * GCOLS
            cols = col0 + np.arange(GCOLS)
            gi, gj = cols // W, cols % W
            in_xy = ((np.abs(gi[None, :] - mx[hit, None]) <= radii[hit, None]) &
                     (np.abs(gj[None, :] - my[hit, None]) <= radii[hit, None]))
            coef_m[10:10 + GCOLS, o:o + nh] = np.where(in_xy, 0.0, PEN).T
            in_z = (np.abs(np.arange(D)[None, :] - mz[hit, None])
                    <= radii[hit, None])
            coef_m[10 + GCOLS:, o:o + nh] = np.where(in_z, 0.0, PEN).T
            sem_m[o:o + nh] = sem[hit].astype(np.float16)
        in_maps.append({"RHS": rhs_all[ci], "COEF": coef_m, "SEM": sem_m})
    return in_maps, L_slots


def kernel(**inputs):
    in_maps, L_slots = _host_prep(**inputs)
    nc = _get_nc(L_slots)
    run = _get_runner(nc)
    results, _, _ = run(in_maps)
    out = np.empty((N, C), np.float32)
    for ci in range(NCORES):
        out[ci * NPC:(ci + 1) * NPC] = results[ci]["OUT"].T
    return out


# revision 8
# speedup vs baseline: 6490.4893x; 6490.4893x over previous
# Trainium2 Bass kernel for nn_LocalAggregator (Gaussian -> voxel-grid semantic
# compositing).  Strategy: data-parallel over the N=129600 query points (8 cores
# x 16200 points).  Each core's slab is split into 45 groups of 360 points (10
# z-columns of 36).  For every group the host builds the exact list of Gaussians
# whose integer box overlaps the group's (x, y) column rectangle; the quadratic
# form, log-opacity, and the x/y/z integer box tests (as one-hot penalty rows)
# are all folded into a single fp32 matmul
#     E[g, n] = coef[56, G].T @ feat[56, 360]
# followed by Exp on the scalar engine and an fp16 matmul with the semantics to
# produce the [13, 360] output block.  No per-pair vector ops at all.
import numpy as np
import ml_dtypes

H, W, D = 60, 60, 36
GRID = 0.08
SCALE_MULT = 3.0
P = 2048
C = 13
N = H * W * D                  # 129600
NCORES = 8
NPC = N // NCORES              # 16200 points per core
GCOLS = 10                     # columns per group
GPTS = GCOLS * D               # 360 points per group
NG = NPC // GPTS               # 45 groups per core
KF = 10 + GCOLS + D            # 56 feature rows
PEN = -2000.0                  # box-miss penalty (exp(PEN) == 0 in fp32)

_NC_CACHE: dict = {}
_JIT_CACHE: dict = {}


def _build_nc(L_slots, use_f32r=False):
    import concourse.bacc as bacc
    import concourse.tile as tile
    from concourse import mybir

    Ltot = int(np.sum(L_slots))
    offs = np.concatenate([[0], np.cumsum(L_slots)]).astype(int)
    units = []
    for g in range(NG):
        L = int(L_slots[g])
        for s in range(0, L, 128):
            units.append((g, int(offs[g]) + s, min(128, L - s)))
    NU = len(units)

    nc = bacc.Bacc("TRN2", target_bir_lowering=False, debug=False,
                   num_devices=NCORES)
    f32 = mybir.dt.float32
    fmm = mybir.dt.float32r if use_f32r else mybir.dt.float32
    f16 = mybir.dt.float16
    RHS = nc.dram_tensor("RHS", [KF, NPC], fmm, kind="ExternalInput")
    COEF = nc.dram_tensor("COEF", [KF, Ltot], fmm, kind="ExternalInput")
    SEMP = nc.dram_tensor("SEMP", [128, NU * C], f16, kind="ExternalInput")
    OUT = nc.dram_tensor("OUT", [C, NPC], f32, kind="ExternalOutput")

    with tile.TileContext(nc) as tc:
        with (
            tc.tile_pool(name="big", bufs=1) as big_pool,
            tc.tile_pool(name="w", bufs=4) as w_pool,
            tc.tile_pool(name="psE", bufs=4, space="PSUM") as pse_pool,
            tc.tile_pool(name="psO", bufs=2, space="PSUM") as pso_pool,
        ):
            rhs_b = big_pool.tile([KF, NPC], fmm)
            coef_b = big_pool.tile([KF, Ltot], fmm)
            semp_b = big_pool.tile([128, NU * C], f16)
            out_b = big_pool.tile([C, NPC], f32)
            # chunked loads so compute can start after the first slice lands
            NCH = 5
            for ch in range(NCH):
                g0, g1 = ch * NG // NCH, (ch + 1) * NG // NCH
                a, b = g0 * GPTS, g1 * GPTS
                nc.sync.dma_start(rhs_b[:, a:b], RHS[:, a:b])
                a, b = int(offs[g0]), int(offs[g1])
                nc.sync.dma_start(coef_b[:, a:b], COEF[:, a:b])
                u0 = sum(1 for (g, _, _) in units if g < g0)
                u1 = sum(1 for (g, _, _) in units if g < g1)
                nc.sync.dma_start(semp_b[:, u0 * C:u1 * C],
                                  SEMP[:, u0 * C:u1 * C])
            ti_prev = -1
            for u, (g, off, Mt) in enumerate(units):
                first = (u == 0) or (units[u - 1][0] != g)
                last = (u == NU - 1) or (units[u + 1][0] != g)
                if first:
                    psO = pso_pool.tile([C, GPTS], f32)
                psE = pse_pool.tile([Mt, GPTS], f32)
                nc.tensor.matmul(psE[:], coef_b[:, off:off + Mt],
                                 rhs_b[:, g * GPTS:(g + 1) * GPTS],
                                 start=True, stop=True)
                w_t = w_pool.tile([Mt, GPTS], f16)
                nc.scalar.activation(w_t[:], psE[:],
                                     mybir.ActivationFunctionType.Exp)
                nc.tensor.matmul(psO[:], semp_b[0:Mt, u * C:(u + 1) * C],
                                 w_t[:], start=first, stop=last)
                if last:
                    nc.vector.tensor_copy(out_b[:, g * GPTS:(g + 1) * GPTS],
                                          psO[:])
            nc.sync.dma_start(OUT[:], out_b[:])
    nc.compile()
    return nc


def _get_nc(L_slots, use_f32r=False):
    key = (tuple(int(x) for x in L_slots), use_f32r)
    if key not in _NC_CACHE:
        _NC_CACHE[key] = _build_nc(L_slots, use_f32r=use_f32r)
    return _NC_CACHE[key]


def _get_runner(nc):
    """Cached shard_map-jitted executor for one Bass program (axon/PJRT path).

    Mirrors concourse.bass2jax.run_bass_via_pjrt but keeps the jitted callable
    so repeated runs don't rebuild/recompile."""
    if id(nc) in _JIT_CACHE:
        return _JIT_CACHE[id(nc)]
    import jax
    from concourse import bass2jax, mybir
    from jax.experimental.shard_map import shard_map
    from jax.sharding import Mesh, PartitionSpec

    bass2jax.install_neuronx_cc_hook()
    partition_name = (nc.partition_id_tensor.name
                      if nc.partition_id_tensor else None)
    in_names, out_names, out_avals, zero_outs = [], [], [], []
    for alloc in nc.m.functions[0].allocations:
        if not isinstance(alloc, mybir.MemoryLocationSet):
            continue
        name = alloc.memorylocations[0].name
        if alloc.kind == "ExternalInput":
            if name == partition_name:
                continue
            in_names.append(name)
        elif alloc.kind == "ExternalOutput":
            shape = tuple(alloc.tensor_shape)
            dtype = mybir.dt.np(alloc.dtype)
            out_names.append(name)
            out_avals.append(jax.core.ShapedArray(shape, dtype))
            zero_outs.append(np.zeros(shape, dtype))
    n_params = len(in_names)
    all_in_names = in_names + out_names
    if partition_name is not None:
        all_in_names = all_in_names + [partition_name]

    def _body(*args):
        operands = list(args)
        if partition_name is not None:
            operands.append(bass2jax.partition_id_tensor())
        outs = bass2jax._bass_exec_p.bind(
            *operands,
            out_avals=tuple(out_avals),
            in_names=tuple(all_in_names),
            out_names=tuple(out_names),
            lowering_input_output_aliases=(),
            sim_require_finite=True,
            sim_require_nnan=True,
            nc=nc,
        )
        return tuple(outs)

    devices = jax.devices()[:NCORES]
    mesh = Mesh(np.asarray(devices), ("core",))
    donate = tuple(range(n_params, n_params + len(out_names)))
    sharded = jax.jit(
        shard_map(_body, mesh=mesh,
                  in_specs=(PartitionSpec("core"),) * (n_params + len(out_names)),
                  out_specs=(PartitionSpec("core"),) * len(out_names),
                  check_rep=False),
        donate_argnums=donate, keep_unused=True)

    def run(in_maps, rounds=1):
        concat_in = [np.concatenate([np.asarray(m[nm]) for m in in_maps], axis=0)
                     for nm in in_names]
        outs = None
        for _ in range(rounds):
            zo = [np.concatenate([z] * NCORES, axis=0) for z in zero_outs]
            outs = sharded(*concat_in, *zo)
        outs = [np.asarray(o) for o in outs]
        results = []
        for ci in range(NCORES):
            d = {}
            for oi, nm in enumerate(out_names):
                per = outs[oi].shape[0] // NCORES
                d[nm] = outs[oi][ci * per:(ci + 1) * per]
            results.append(d)
        return results, sharded, (concat_in, zero_outs, in_names, out_names)

    sharded_nd = jax.jit(
        shard_map(_body, mesh=mesh,
                  in_specs=(PartitionSpec("core"),) * (n_params + len(out_names)),
                  out_specs=(PartitionSpec("core"),) * len(out_names),
                  check_rep=False),
        keep_unused=True)

    def timeit(in_maps, iters=30):
        import time as _time
        from jax.sharding import NamedSharding
        sh = NamedSharding(mesh, PartitionSpec("core"))
        concat_in = [np.concatenate([np.asarray(m[nm]) for m in in_maps], axis=0)
                     for nm in in_names]
        zo = [np.concatenate([z] * NCORES, axis=0) for z in zero_outs]
        args = [jax.device_put(a, sh) for a in concat_in + zo]
        outs = sharded_nd(*args)
        jax.block_until_ready(outs)
        t0 = _time.time()
        for _ in range(iters):
            outs = sharded_nd(*args)
        jax.block_until_ready(outs)
        return (_time.time() - t0) / iters

    run.timeit = timeit
    _JIT_CACHE[id(nc)] = run
    return run


def _host_prep(pts, means3D, opacities, semantics, scales, cov3D, origin_use):
    pts = np.asarray(pts, np.float32).reshape(N, 3)
    mu32 = np.asarray(means3D, np.float32).reshape(P, 3)
    op = np.asarray(opacities, np.float64).reshape(P)
    sem = np.asarray(semantics, np.float32).reshape(P, C)
    sc32 = np.asarray(scales, np.float32).reshape(P, 3)
    cov = np.asarray(cov3D, np.float64).reshape(P, 3, 3)
    org32 = np.asarray(origin_use, np.float32).reshape(3)

    # --- integer binning, replicated in fp32 exactly like the reference ---
    radii = np.ceil(sc32.max(-1) * np.float32(SCALE_MULT) / np.float32(GRID)
                    ).astype(np.int32).astype(np.int64)
    m_int = ((mu32 - org32) / np.float32(GRID)).astype(np.int32).astype(np.int64)
    p_int = ((pts - org32) / np.float32(GRID)).astype(np.int32).astype(np.int64)

    # structured-input check: points must be the (i, j, k) voxel-center grid
    idx = np.arange(N)
    kk = idx % D
    col = idx // D
    jj = col % W
    ii = col // W
    grid_int = np.stack([ii, jj, kk], axis=-1)
    if not np.array_equal(p_int, grid_int):
        raise RuntimeError("kernel: unstructured pts not supported by fast path")

    # --- per-Gaussian E coefficients (float64 for the inverse, cast to f32) ---
    a, b, c_, d, e, f = (cov[:, 0, 0], cov[:, 1, 1], cov[:, 2, 2],
                         cov[:, 0, 1], cov[:, 1, 2], cov[:, 0, 2])
    det = a * (b * c_ - e * e) - d * (d * c_ - e * f) + f * (d * e - b * f)
    ixx = (b * c_ - e * e) / det
    iyy = (a * c_ - f * f) / det
    izz = (a * b - d * d) / det
    ixy = (e * f - d * c_) / det
    iyz = (d * f - a * e) / det
    ixz = (d * e - b * f) / det
    A = np.empty((P, 3, 3))
    A[:, 0, 0], A[:, 1, 1], A[:, 2, 2] = ixx, iyy, izz
    A[:, 0, 1] = A[:, 1, 0] = ixy
    A[:, 1, 2] = A[:, 2, 1] = iyz
    A[:, 0, 2] = A[:, 2, 0] = ixz
    mu = mu32.astype(np.float64)
    Amu = np.einsum('pij,pj->pi', A, mu)
    muAmu = np.einsum('pi,pi->p', mu, Amu)
    coef10 = np.stack([
        -0.5 * ixx, -0.5 * iyy, -0.5 * izz,
        -ixy, -iyz, -ixz,
        Amu[:, 0], Amu[:, 1], Amu[:, 2],
        -0.5 * muAmu + np.log(op),
    ]).astype(np.float32)                                    # [10, P]

    # --- per-core RHS feature matrices ---
    x, y, z = pts[:, 0], pts[:, 1], pts[:, 2]
    feat10 = np.stack([x * x, y * y, z * z, x * y, y * z, x * z,
                       x, y, z, np.ones_like(x)])            # [10, N] f32
    nloc = np.arange(NPC)
    cg = (nloc % GPTS) // D
    kz = nloc % D
    onehot = np.zeros((GCOLS + D, NPC), np.float32)
    onehot[cg, nloc] = 1.0
    onehot[GCOLS + kz, nloc] = 1.0
    rhs_all = []
    for ci in range(NCORES):
        r = np.empty((KF, NPC), np.float32)
        r[:10] = feat10[:, ci * NPC:(ci + 1) * NPC]
        r[10:] = onehot
        rhs_all.append(r)

    # --- per-(core, group) Gaussian hit lists and penalty tables ---
    mx, my, mz = m_int[:, 0], m_int[:, 1], m_int[:, 2]
    hits_cg = [[None] * NG for _ in range(NCORES)]
    for ci in range(NCORES):
        for g in range(NG):
            col0 = ci * (NPC // D) + g * GCOLS
            cols = col0 + np.arange(GCOLS)
            gi, gj = cols // W, cols % W
            ox = (mx + radii >= gi.min()) & (mx - radii <= gi.max())
            oy = (my + radii >= gj.min()) & (my - radii <= gj.max())
            hits_cg[ci][g] = np.where(ox & oy)[0]
    L_slots = [max(1, max(len(hits_cg[ci][g]) for ci in range(NCORES)))
               for g in range(NG)]
    Ltot = int(np.sum(L_slots))
    offs = np.concatenate([[0], np.cumsum(L_slots)]).astype(int)

    units = []
    for g in range(NG):
        L = int(L_slots[g])
        for s in range(0, L, 128):
            units.append((g, int(offs[g]) + s, min(128, L - s)))
    NU = len(units)

    in_maps = []
    for ci in range(NCORES):
        coef_m = np.zeros((KF, Ltot), np.float32)
        sem_m = np.zeros((Ltot, C), np.float16)
        for g in range(NG):
            hit = hits_cg[ci][g]
            nh = len(hit)
            if nh == 0:
                continue
            o = offs[g]
            coef_m[:10, o:o + nh] = coef10[:, hit]
            col0 = ci * (NPC // D) + g * GCOLS
            cols = col0 + np.arange(GCOLS)
            gi, gj = cols // W, cols % W
            in_xy = ((np.abs(gi[None, :] - mx[hit, None]) <= radii[hit, None]) &
                     (np.abs(gj[None, :] - my[hit, None]) <= radii[hit, None]))
            coef_m[10:10 + GCOLS, o:o + nh] = np.where(in_xy, 0.0, PEN).T
            in_z = (np.abs(np.arange(D)[None, :] - mz[hit, None])
                    <= radii[hit, None])
            coef_m[10 + GCOLS:, o:o + nh] = np.where(in_z, 0.0, PEN).T
            sem_m[o:o + nh] = sem[hit].astype(np.float16)
        semp = np.zeros((128, NU * C), np.float16)
        for u, (g, off, Mt) in enumerate(units):
            semp[0:Mt, u * C:(u + 1) * C] = sem_m[off:off + Mt]
        in_maps.append({"RHS": rhs_all[ci], "COEF": coef_m, "SEMP": semp})
    return in_maps, L_slots


def kernel(**inputs):
    in_maps, L_slots = _host_prep(**inputs)
    nc = _get_nc(L_slots)
    run = _get_runner(nc)
    results, _, _ = run(in_maps)
    out = np.empty((N, C), np.float32)
    for ci in range(NCORES):
        out[ci * NPC:(ci + 1) * NPC] = results[ci]["OUT"].T
    return out
